# revision 4
# baseline (speedup 1.0000x reference)
"""Trainium2 Bass kernel for nn_Attention_86268713108190.

7 independent attention "bands" over batch 8, n=512, d=512, 8 heads,
shared Wqkv/Wout. Sharding: data-parallel over batch — core c handles
batch index c (7 band-samples of [512, 512] each).

Per-core dataflow (per sample, all matmuls in float32r):
  1. qkvT = Wqkv @ x^T    (lhsT = WqkvT chunks, rhs = x^T)      [e, n]
  2. v    = x @ Wv^T      (lhsT = x^T chunks,   rhs = WvT)      [n, ev]
     v_aug: per head 64 v-cols + a ones column (65) -> denominator for free
  3. per head: S^T = k_h q_h^T (K=64, two heads packed via tile_position),
     expS^T = exp(SCALE*S^T) on ACT (PSUM->SBUF, rounds to f32r),
     O_aug^T[65, n] = v_aug^T-style matmul accumulating over j;
     row 64 = softmax denominator.
  4. recip denominators (DVE), partition-broadcast (GpSimd),
     normalize O^T (DVE) -> OT in [d, n] layout.
  5. out = O @ Wout^T + bias  (lhsT = OT chunks, rhs = WoutT).
"""

import sys

if '/opt/trn_rl_repo' not in sys.path:
    sys.path.insert(0, '/opt/trn_rl_repo')

import numpy as np

P = 128
NSEQ = 512
D = 512
H = 8
DH = 64
NBANDS = 7
NCORES = 8
SCALE = D ** -0.5

_cached = None


def build_kernel(nbands=NBANDS):
    import concourse.bass as bass  # noqa: F401
    import concourse.mybir as mybir
    import concourse.tile as tile
    from concourse import bacc
    from concourse import library_config

    f32 = mybir.dt.float32
    f32r = mybir.dt.float32r
    Exp = mybir.ActivationFunctionType.Exp

    nc = bacc.Bacc("TRN2", target_bir_lowering=False, debug=False,
                   num_devices=NCORES)

    xT = nc.dram_tensor("xT", [nbands, D, NSEQ], f32r, kind="ExternalInput").ap()
    wqkvT = nc.dram_tensor("wqkvT", [D, 3 * D], f32r, kind="ExternalInput").ap()
    woutT = nc.dram_tensor("woutT", [D, D], f32r, kind="ExternalInput").ap()
    biasb = nc.dram_tensor("biasb", [P, D], f32, kind="ExternalInput").ap()
    out = nc.dram_tensor("out", [nbands, NSEQ, D], f32, kind="ExternalOutput").ap()

    nc.gpsimd.load_library(library_config.attn)

    with tile.TileContext(nc) as tc:
        with (
            tc.tile_pool(name="weights", bufs=1) as wpool,
            tc.tile_pool(name="x", bufs=2) as xpool,
            tc.tile_pool(name="qk", bufs=2) as qkpool,
            tc.tile_pool(name="v", bufs=2) as vpool,
            tc.tile_pool(name="ot", bufs=2) as otpool,
            tc.tile_pool(name="es", bufs=3) as spool,
            tc.tile_pool(name="r", bufs=3) as rpool,
            tc.tile_pool(name="ob", bufs=3) as outpool,
            tc.tile_pool(name="psproj", bufs=2, space="PSUM") as psproj,
            tc.tile_pool(name="pss", bufs=2, space="PSUM") as pss,
            tc.tile_pool(name="pso", bufs=2, space="PSUM") as pso,
        ):
            wq_sb = wpool.tile([P, 4, 3 * D], f32r)
            wo_sb = wpool.tile([P, 4, D], f32r)
            bias_sb = wpool.tile([P, D], f32)
            nc.sync.dma_start(wq_sb[:], wqkvT.rearrange("(ko ki) e -> ki ko e", ki=P))
            nc.sync.dma_start(wo_sb[:], woutT.rearrange("(ko ki) e -> ki ko e", ki=P))
            nc.sync.dma_start(bias_sb[:], biasb[:])

            for s in range(nbands):
                xt = xpool.tile([P, 4, NSEQ], f32r, tag="xt")
                nc.sync.dma_start(
                    xt[:], xT[s].rearrange("(ko ki) n -> ki ko n", ki=P))

                # --- QKV projections -> qkvT layout for q,k ---
                qk_sb = qkpool.tile([P, 8, NSEQ], f32r, tag="qk")
                for et in range(8):
                    ps = psproj.tile([P, NSEQ], f32, tag="psproj")
                    for kt in range(4):
                        nc.tensor.matmul(
                            ps[:],
                            wq_sb[:, kt, et * P:(et + 1) * P],
                            xt[:, kt, :],
                            start=(kt == 0), stop=(kt == 3))
                    nc.vector.tensor_copy(qk_sb[:, et, :], ps[:])

                # --- V projection -> row-major v_aug with ones column ---
                v_aug = vpool.tile([P, 4, H, DH + 1], f32r, tag="vaug")
                for nt in range(4):
                    ps = psproj.tile([P, NSEQ], f32, tag="psproj")
                    for kt in range(4):
                        nc.tensor.matmul(
                            ps[:],
                            xt[:, kt, nt * P:(nt + 1) * P],
                            wq_sb[:, kt, 2 * D:3 * D],
                            start=(kt == 0), stop=(kt == 3))
                    nc.vector.tensor_copy(
                        v_aug[:, nt, :, 0:DH],
                        ps[:].rearrange("p (h dh) -> p h dh", h=H))
                    nc.vector.memset(v_aug[:, nt, :, DH:DH + 1].bitcast(f32), 1.0)

                # --- attention per head pair (2g, 2g+1) ---
                ot_sb = otpool.tile([P, 4, NSEQ], f32r, tag="ot")
                for g in range(4):
                    ps_o0 = pso.tile([DH + 1, NSEQ], f32, tag="pso")
                    ps_o1 = pso.tile([DH + 1, NSEQ], f32, tag="pso")
                    for jt in range(4):
                        ps_s = pss.tile([P, 2, NSEQ], f32, tag="pss")
                        nc.tensor.matmul(
                            ps_s[:, 0, :],
                            qk_sb[0:DH, 4 + g, jt * P:(jt + 1) * P],
                            qk_sb[0:DH, g, :],
                            start=True, stop=True)
                        nc.tensor.matmul(
                            ps_s[:, 1, :],
                            qk_sb[DH:P, 4 + g, jt * P:(jt + 1) * P],
                            qk_sb[DH:P, g, :],
                            start=True, stop=True, tile_position=(DH, 0))
                        es = spool.tile([P, 2, NSEQ], f32r, tag="es")
                        nc.scalar.activation(es[:], ps_s[:], Exp, scale=SCALE)
                        nc.tensor.matmul(
                            ps_o0[:], v_aug[:, jt, 2 * g, :], es[:, 0, :],
                            start=(jt == 0), stop=(jt == 3))
                        nc.tensor.matmul(
                            ps_o1[:], v_aug[:, jt, 2 * g + 1, :], es[:, 1, :],
                            start=(jt == 0), stop=(jt == 3))
                    rc0 = rpool.tile([1, NSEQ], f32, tag="rc0")
                    rc1 = rpool.tile([1, NSEQ], f32, tag="rc1")
                    nc.vector.reciprocal(rc0[:], ps_o0[DH:DH + 1, :])
                    nc.vector.reciprocal(rc1[:], ps_o1[DH:DH + 1, :])
                    rb0 = rpool.tile([DH, NSEQ], f32, tag="rb0")
                    rb1 = rpool.tile([DH, NSEQ], f32, tag="rb1")
                    nc.gpsimd.partition_broadcast(rb0[:], rc0[:])
                    nc.gpsimd.partition_broadcast(rb1[:], rc1[:])
                    nc.vector.tensor_mul(
                        ot_sb[0:DH, g, :], ps_o0[0:DH, :], rb0[:])
                    nc.vector.tensor_mul(
                        ot_sb[DH:P, g, :], ps_o1[0:DH, :], rb1[:])

                # --- output projection + bias ---
                for nt in range(4):
                    ps = psproj.tile([P, NSEQ], f32, tag="psproj")
                    for kt in range(4):
                        nc.tensor.matmul(
                            ps[:],
                            ot_sb[:, kt, nt * P:(nt + 1) * P],
                            wo_sb[:, kt, :],
                            start=(kt == 0), stop=(kt == 3))
                    ob = outpool.tile([P, D], f32, tag="ob")
                    nc.vector.tensor_add(ob[:], ps[:], bias_sb[:])
                    nc.sync.dma_start(
                        out[s].rearrange("(no ni) e -> ni no e", ni=P)[:, nt, :],
                        ob[:])

    nc.compile()
    return nc


def _get_nc():
    global _cached
    if _cached is None:
        _cached = build_kernel()
    return _cached


def make_in_maps(x, x_delta, x_theta, x_alpha, x_beta, x_gamma, x_upper,
                 Wqkv, Wout, bout):
    xs = np.stack([np.asarray(a, dtype=np.float32) for a in
                   (x, x_delta, x_theta, x_alpha, x_beta, x_gamma, x_upper)],
                  axis=0)  # [7, b, n, d]
    xsT = np.ascontiguousarray(xs.transpose(1, 0, 3, 2))  # [b, 7, d, n]
    wqkvT = np.ascontiguousarray(np.asarray(Wqkv, np.float32).T)  # [d, 3d]
    woutT = np.ascontiguousarray(np.asarray(Wout, np.float32).T)  # [d, d]
    biasb = np.ascontiguousarray(
        np.broadcast_to(np.asarray(bout, np.float32)[None, :], (P, D)))
    return [
        {"xT": xsT[c], "wqkvT": wqkvT, "woutT": woutT, "biasb": biasb}
        for c in range(NCORES)
    ]


def kernel(x, x_delta, x_theta, x_alpha, x_beta, x_gamma, x_upper,
           Wqkv, Wout, bout):
    from concourse.bass_utils import run_bass_kernel_spmd

    nc = _get_nc()
    in_maps = make_in_maps(x, x_delta, x_theta, x_alpha, x_beta, x_gamma,
                           x_upper, Wqkv, Wout, bout)
    res = run_bass_kernel_spmd(nc, in_maps, core_ids=list(range(NCORES)))
    full = np.empty((NBANDS, NCORES, NSEQ, D), dtype=np.float32)
    for c in range(NCORES):
        full[:, c] = res.results[c]["out"]
    return tuple(full[i] for i in range(NBANDS))
